# revision 30
# baseline (speedup 1.0000x reference)
"""v8: all-bf16 matmuls, permuted head layout, pipelined attention.

vs v6 fp32r baseline (444us):
- All matmul operands bf16: separate LDWEIGHTS (FWL + PE pull-ahead) vs
  fp32r's serial self-loading; row-disjoint matmuls run concurrently on
  the PE sub-arrays.
- q/k feature rows permuted to [h0.lo | h1.lo | h0.hi | h1.hi] (32 each):
  rope's rotate-half becomes 2 DVE muls (+-64 partition shift) instead of
  4, and scores run as 4 concurrent K=32 row-tiled matmuls per k-tile.
- V stored [128, ktile, head, 1+64] with the softmax-denominator ones
  column FIRST, so the reciprocal chain reads partition 0 (no shifts).
- x converted to bf16 on host, streamed once, kept resident in SBUF.
- exp owns ScalarE (~200us floor); attention av matmuls trail their exp
  by 2 k-tile-pairs; pav is copied to SBUF immediately after the last av
  so the next chunk's av(start) isn't gated on the normalize chain.
"""

import os
import sys

for _p in ("/opt/trn_rl_repo", "/root/.axon_site/_ro/trn_rl_repo"):
    if os.path.isdir(_p) and _p not in sys.path:
        sys.path.insert(0, _p)

import contextlib

import ml_dtypes
import numpy as np

import concourse.bass as bass
import concourse.tile as tile
from concourse import bacc, mybir
from concourse.bass_utils import run_bass_kernel_spmd

P = 128
L = 2048
D = 1536
HL = 6           # heads per core
HD = 64
EQ = 384         # HL * HD
DC = D // P      # 12
LT = L // P      # 16
TP = LT // 2     # 8 k-tile pairs
ACH = 512        # attention q chunk
XCH = 512        # qkv l chunk
NCH = L // XCH   # 4
F32 = mybir.dt.float32
BF16 = mybir.dt.bfloat16
F16 = mybir.dt.float16
AF = mybir.ActivationFunctionType

DEBUG_DUMP = os.environ.get("DEBUG_DUMP", "0") == "1"


def build_bass(repeat=1):
    nc = bacc.Bacc("TRN2", target_bir_lowering=False, debug=False, num_devices=8)
    xT = nc.dram_tensor("xT", [D, L], BF16, kind="ExternalInput")
    wqkT = nc.dram_tensor("wqkT", [D, 768], BF16, kind="ExternalInput")
    wvT = nc.dram_tensor("wvT", [D, EQ], BF16, kind="ExternalInput")
    woT = nc.dram_tensor("woT", [EQ, D], BF16, kind="ExternalInput")
    cos2 = nc.dram_tensor("cos2", [P, L], F16, kind="ExternalInput")
    ss2 = nc.dram_tensor("ss2", [P, L], F16, kind="ExternalInput")
    out = nc.dram_tensor("out", [L, D], F32, kind="ExternalOutput")
    if DEBUG_DUMP:
        dbg_qT = nc.dram_tensor("dbg_qT", [P, 3 * L], BF16, kind="ExternalOutput")
        dbg_kT = nc.dram_tensor("dbg_kT", [P, 3 * L], BF16, kind="ExternalOutput")
        dbg_oT = nc.dram_tensor("dbg_oT", [P, 3 * L], BF16, kind="ExternalOutput")

    xT_r = xT.rearrange("(dc p) l -> p dc l", p=P)
    wqkT_r = wqkT.rearrange("(dc p) e -> p dc e", p=P)
    wvT_r = wvT.rearrange("(dc p) e -> p dc e", p=P)
    woT_r = woT.rearrange("(ec p) d -> p ec d", p=P)

    with tile.TileContext(nc) as tc:
        rep_cm = tc.For_i(0, repeat, 1) if repeat > 1 else contextlib.nullcontext()
        with rep_cm, tc.tile_pool(name="persist", bufs=1) as persist:
            qT = persist.tile([P, 3, L], BF16)
            kT = persist.tile([P, 3, L], BF16)
            outT = persist.tile([P, 3, L], BF16)
            # [P, ktile, head, 64 dims + ones (softmax denominator)]
            v1 = persist.tile([P, LT, HL, HD + 1], BF16)
            xsb = persist.tile([P, DC, L], BF16)
            cos_sb = persist.tile([P, L], F16)
            ss_sb = persist.tile([P, L], F16)
            wqk_sb = persist.tile([P, DC, 768], BF16)
            wv_sb = persist.tile([P, DC, EQ], BF16)
            wo_sb = persist.tile([P, 3, D], BF16)

            ones_c = nc.const_aps.tensor(1.0, (P, 1), F32)
            nc.vector.tensor_copy(
                v1[:, :, :, HD : HD + 1], ones_c.to_broadcast([P, LT, HL, 1])
            )
            nc.sync.dma_start(cos_sb[:], cos2[:])
            nc.sync.dma_start(ss_sb[:], ss2[:])
            for d0 in range(0, DC, 2):
                nc.sync.dma_start(
                    wqk_sb[:, d0 : d0 + 2, :], wqkT_r[:, d0 : d0 + 2, :]
                )
            for d0 in range(0, DC, 4):
                nc.sync.dma_start(wv_sb[:, d0 : d0 + 4, :], wvT_r[:, d0 : d0 + 4, :])
            nc.sync.dma_start(wo_sb[:], woT_r[:])

            with (
                tc.tile_pool(name="s2t", bufs=3) as s2t,
                tc.tile_pool(name="s2att", bufs=4) as s2att,
                tc.tile_pool(name="s2o", bufs=3) as s2o,
                tc.tile_pool(name="s2nrm", bufs=4) as s2nrm,
                tc.tile_pool(name="ps_acc", bufs=2, space=bass.MemorySpace.PSUM) as ps_acc,
                tc.tile_pool(name="ps_s", bufs=2, space=bass.MemorySpace.PSUM) as ps_s,
                tc.tile_pool(name="ps_av", bufs=2, space=bass.MemorySpace.PSUM) as ps_av,
            ):

                def qk_chunk(etp, c, with_dma):
                    sl = slice(c * XCH, (c + 1) * XCH)
                    if with_dma:
                        for d0 in range(0, DC, 3):
                            nc.sync.dma_start(
                                xsb[:, d0 : d0 + 3, sl], xT_r[:, d0 : d0 + 3, sl]
                            )
                    for half in range(2):  # 0: q, 1: k
                        ps = ps_acc.tile([P, XCH], F32, tag="acc")
                        wcol = etp * 256 + half * P
                        for dc in range(DC):
                            nc.tensor.matmul(
                                ps[:],
                                wqk_sb[:, dc, wcol : wcol + P],
                                xsb[:, dc, sl],
                                start=(dc == 0),
                                stop=(dc == DC - 1),
                            )
                        # rope: PSUM-direct muls (partition-shifted reads
                        # are only legal with a PSUM operand)
                        tcos = s2t.tile([P, XCH], BF16, tag="tcos")
                        trot = s2t.tile([P, XCH], BF16, tag="trot")
                        nc.vector.tensor_mul(tcos[:], ps[:], cos_sb[:, sl])
                        for q_ in range(4):
                            s = (q_ ^ 1) * 32
                            d_ = q_ * 32
                            nc.vector.tensor_mul(
                                trot[d_ : d_ + 32, :],
                                ps[s : s + 32, :],
                                ss_sb[d_ : d_ + 32, sl],
                            )
                        dst = (qT if half == 0 else kT)[:, etp, sl]
                        nc.gpsimd.tensor_add(dst, tcos[:], trot[:])

                def v_chunk(etp, c):
                    # after attention(etp-1, c): ACT copies rank behind that
                    # unit's exps; v1 is only read one unit later
                    for lt2 in range(XCH // P):
                        lk = c * (XCH // P) + lt2
                        lsl = slice(c * XCH + lt2 * P, c * XCH + (lt2 + 1) * P)
                        pv = ps_acc.tile([P, XCH], F32, tag="acc")
                        for dc in range(DC):
                            nc.tensor.matmul(
                                pv[:, 0 : 2 * HD],
                                xsb[:, dc, lsl],
                                wv_sb[:, dc, etp * 2 * HD : (etp + 1) * 2 * HD],
                                start=(dc == 0),
                                stop=(dc == DC - 1),
                            )
                        nc.scalar.copy(
                            v1[:, lk, 2 * etp : 2 * etp + 2, 0:HD],
                            pv[:, 0 : 2 * HD].rearrange("p (h d) -> p h d", h=2),
                        )

                def attention(etp, cq):
                    cqs = slice(cq * ACH, (cq + 1) * ACH)
                    pav0 = ps_av.tile([HD + 1, ACH], F32, tag="av")
                    pav1 = ps_av.tile([HD + 1, ACH], F32, tag="av")
                    atts = {}

                    def emit_av(t):
                        for hh, pav in ((0, pav0), (1, pav1)):
                            for i in range(2):
                                lk = 2 * t + i
                                nc.tensor.matmul(
                                    pav[:],
                                    v1[:, lk, 2 * etp + hh, :],
                                    atts[t][:, i, hh, :],
                                    start=(lk == 0),
                                    stop=(lk == LT - 1),
                                )

                    for t in range(TP):
                        att = s2att.tile([P, 2, 2, ACH], BF16, tag="att")
                        for i in range(2):
                            lk = 2 * t + i
                            psc = ps_s.tile([P, 2 * ACH], F32, tag="s")
                            for hh in range(2):
                                po = hh * HD
                                nc.tensor.matmul(
                                    psc[:, hh * ACH : (hh + 1) * ACH],
                                    kT[po : po + HD, etp, lk * P : (lk + 1) * P],
                                    qT[po : po + HD, etp, cqs],
                                    start=True,
                                    stop=True,
                                )
                            nc.scalar.activation(
                                att[:, i, :, :].rearrange("p h a -> p (h a)"),
                                psc[:],
                                AF.Exp,
                                scale=0.125,
                            )
                        atts[t] = att
                        # av two tpairs behind its exp: PE never waits ScalarE
                        if t > 1:
                            emit_av(t - 2)
                    emit_av(TP - 2)
                    emit_av(TP - 1)

                    # drain pav to SBUF immediately (DVE), freeing the PSUM
                    # bank for the next chunk's av(start); the reciprocal /
                    # broadcast / scale run off the critical path (gpsimd)
                    for hh, pav in ((0, pav0), (1, pav1)):
                        po = hh * HD
                        pavs = s2nrm.tile([HD, ACH], F32, tag="pavs")
                        nc.vector.tensor_copy(pavs[:], pav[0:HD, :])
                        dcp = s2nrm.tile([1, ACH], F32, tag="dcp")
                        nc.vector.tensor_copy(dcp[:], pav[HD : HD + 1, :])
                        rcp = s2nrm.tile([1, ACH], F32, tag="rcp")
                        nc.vector.reciprocal_approx_fast(out=rcp[:], in_=dcp[:])
                        rb = s2nrm.tile([HD, ACH], F32, tag="rb")
                        nc.gpsimd.partition_broadcast(rb[:], rcp[:], channels=HD)
                        nc.gpsimd.tensor_mul(
                            outT[po : po + HD, etp, cqs], pavs[:], rb[:]
                        )

                def oproj(cq):
                    for lt in range(ACH // P):
                        l0 = cq * ACH + lt * P
                        for dn in range(D // ACH):
                            pso = ps_acc.tile([P, ACH], F32, tag="acc")
                            for ec in range(3):
                                nc.tensor.matmul(
                                    pso[:],
                                    outT[:, ec, l0 : l0 + P],
                                    wo_sb[:, ec, dn * ACH : (dn + 1) * ACH],
                                    start=(ec == 0),
                                    stop=(ec == 2),
                                )
                            ot = s2o.tile([P, ACH], F32)
                            nc.vector.tensor_copy(ot[:], pso[:])
                            nc.sync.dma_start(
                                out[l0 : l0 + P, dn * ACH : (dn + 1) * ACH], ot[:]
                            )

                # phase A: qkv+v for pair 0 (x DMA'd once, stays resident)
                for c in range(NCH):
                    qk_chunk(0, c, with_dma=True)
                    v_chunk(0, c)
                # steady state: attention(etp) overlapped with qkv(etp+1)
                for etp in range(3):
                    for cq in range(L // ACH):
                        if etp < 2:
                            qk_chunk(etp + 1, cq, with_dma=False)
                        attention(etp, cq)
                        if etp < 2:
                            v_chunk(etp + 1, cq)
                        if etp == 2:
                            oproj(cq)

                if DEBUG_DUMP:
                    nc.sync.dma_start(
                        dbg_qT[:], qT[:].rearrange("p a b -> p (a b)")
                    )
                    nc.sync.dma_start(
                        dbg_kT[:], kT[:].rearrange("p a b -> p (a b)")
                    )
                    nc.sync.dma_start(
                        dbg_oT[:], outT[:].rearrange("p a b -> p (a b)")
                    )

    nc.compile()
    return nc


_NC_CACHE = None


def _get_nc():
    global _NC_CACHE
    if _NC_CACHE is None:
        _NC_CACHE = build_bass()
    return _NC_CACHE


def make_in_maps(x, w_qkv, w_o, cos, sin):
    x = np.asarray(x, dtype=np.float32)
    w_qkv = np.asarray(w_qkv, dtype=np.float32)
    w_o = np.asarray(w_o, dtype=np.float32)
    cos = np.asarray(cos, dtype=np.float32)
    sin = np.asarray(sin, dtype=np.float32)
    bf16 = ml_dtypes.bfloat16

    cosT = np.ascontiguousarray(cos.T)
    sinT = sin.T
    ss = np.concatenate([-sinT[0:32], sinT[32:64]], axis=0)
    cos2 = np.ascontiguousarray(np.tile(cosT, (2, 1))).astype(np.float16)
    ss2 = np.ascontiguousarray(np.tile(ss, (2, 1))).astype(np.float16)

    def perm_block(w2h):
        return w2h

    in_maps = []
    for c in range(8):
        b, g = c // 4, c % 4
        xTc = np.ascontiguousarray(x[b].T).astype(bf16)
        wq = w_qkv[g * EQ : (g + 1) * EQ]
        wk = w_qkv[D + g * EQ : D + (g + 1) * EQ]
        wv = w_qkv[2 * D + g * EQ : 2 * D + (g + 1) * EQ]
        blocks = []
        for etp in range(3):
            blocks.append(perm_block(wq[etp * 128 : (etp + 1) * 128]))
            blocks.append(perm_block(wk[etp * 128 : (etp + 1) * 128]))
        wqkTc = np.ascontiguousarray(np.concatenate(blocks, 0).T).astype(bf16)
        wvTc = np.ascontiguousarray(wv.T).astype(bf16)
        woTc = np.ascontiguousarray(w_o[:, g * EQ : (g + 1) * EQ].T).astype(bf16)
        in_maps.append(
            {
                "xT": xTc,
                "wqkT": wqkTc,
                "wvT": wvTc,
                "woT": woTc,
                "cos2": cos2,
                "ss2": ss2,
            }
        )
    return in_maps


def kernel(x, w_qkv, w_o, cos, sin):
    nc = _get_nc()
    in_maps = make_in_maps(x, w_qkv, w_o, cos, sin)
    res = run_bass_kernel_spmd(nc, in_maps, core_ids=list(range(8)))
    outs = [res.results[c]["out"] for c in range(8)]
    full = np.stack(
        [
            outs[0] + outs[1] + outs[2] + outs[3],
            outs[4] + outs[5] + outs[6] + outs[7],
        ]
    ).astype(np.float32)
    return full


# revision 31
# speedup vs baseline: 1.2784x; 1.2784x over previous
"""v8: all-bf16 matmuls, permuted head layout, pipelined attention.

vs v6 fp32r baseline (444us):
- All matmul operands bf16: separate LDWEIGHTS (FWL + PE pull-ahead) vs
  fp32r's serial self-loading; row-disjoint matmuls run concurrently on
  the PE sub-arrays.
- q/k feature rows permuted to [h0.lo | h1.lo | h0.hi | h1.hi] (32 each):
  rope's rotate-half becomes 2 DVE muls (+-64 partition shift) instead of
  4, and scores run as 4 concurrent K=32 row-tiled matmuls per k-tile.
- V stored [128, ktile, head, 1+64] with the softmax-denominator ones
  column FIRST, so the reciprocal chain reads partition 0 (no shifts).
- x converted to bf16 on host, streamed once, kept resident in SBUF.
- exp owns ScalarE (~200us floor); attention av matmuls trail their exp
  by 2 k-tile-pairs; pav is copied to SBUF immediately after the last av
  so the next chunk's av(start) isn't gated on the normalize chain.
"""

import os
import sys

for _p in ("/opt/trn_rl_repo", "/root/.axon_site/_ro/trn_rl_repo"):
    if os.path.isdir(_p) and _p not in sys.path:
        sys.path.insert(0, _p)

import contextlib

import ml_dtypes
import numpy as np

import concourse.bass as bass
import concourse.tile as tile
from concourse import bacc, mybir
from concourse.bass_utils import run_bass_kernel_spmd

P = 128
L = 2048
D = 1536
HL = 6           # heads per core
HD = 64
EQ = 384         # HL * HD
DC = D // P      # 12
LT = L // P      # 16
TP = LT // 2     # 8 k-tile pairs
ACH = 512        # attention q chunk
XCH = 512        # qkv l chunk
NCH = L // XCH   # 4
F32 = mybir.dt.float32
BF16 = mybir.dt.bfloat16
F16 = mybir.dt.float16
AF = mybir.ActivationFunctionType

DEBUG_DUMP = os.environ.get("DEBUG_DUMP", "0") == "1"


def build_bass(repeat=1):
    nc = bacc.Bacc("TRN2", target_bir_lowering=False, debug=False, num_devices=8)
    xT = nc.dram_tensor("xT", [D, L], BF16, kind="ExternalInput")
    wqkT = nc.dram_tensor("wqkT", [D, 768], BF16, kind="ExternalInput")
    wvT = nc.dram_tensor("wvT", [D, EQ], BF16, kind="ExternalInput")
    woT = nc.dram_tensor("woT", [EQ, D], BF16, kind="ExternalInput")
    cos2 = nc.dram_tensor("cos2", [P, L], F16, kind="ExternalInput")
    ss2 = nc.dram_tensor("ss2", [P, L], F16, kind="ExternalInput")
    out = nc.dram_tensor("out", [L, D], F32, kind="ExternalOutput")
    if DEBUG_DUMP:
        dbg_qT = nc.dram_tensor("dbg_qT", [P, 3 * L], BF16, kind="ExternalOutput")
        dbg_kT = nc.dram_tensor("dbg_kT", [P, 3 * L], BF16, kind="ExternalOutput")
        dbg_oT = nc.dram_tensor("dbg_oT", [P, 3 * L], BF16, kind="ExternalOutput")

    xT_r = xT.rearrange("(dc p) l -> p dc l", p=P)
    wqkT_r = wqkT.rearrange("(dc p) e -> p dc e", p=P)
    wvT_r = wvT.rearrange("(dc p) e -> p dc e", p=P)
    woT_r = woT.rearrange("(ec p) d -> p ec d", p=P)

    with tile.TileContext(nc) as tc:
        rep_cm = tc.For_i(0, repeat, 1) if repeat > 1 else contextlib.nullcontext()
        with rep_cm, tc.tile_pool(name="persist", bufs=1) as persist:
            qT = persist.tile([P, 3, L], BF16)
            kT = persist.tile([P, 3, L], BF16)
            outT = persist.tile([P, 3, L], BF16)
            # [P, ktile, head, 64 dims + ones (softmax denominator)]
            v1 = persist.tile([P, LT, HL, HD + 1], BF16)
            xsb = persist.tile([P, DC, L], BF16)
            cos_sb = persist.tile([P, L], F16)
            ss_sb = persist.tile([P, L], F16)
            wqk_sb = persist.tile([P, DC, 768], BF16)
            wv_sb = persist.tile([P, DC, EQ], BF16)
            wo_sb = persist.tile([P, 3, D], BF16)

            ones_c = nc.const_aps.tensor(1.0, (P, 1), F32)
            nc.vector.tensor_copy(
                v1[:, :, :, HD : HD + 1], ones_c.to_broadcast([P, LT, HL, 1])
            )
            nc.sync.dma_start(cos_sb[:], cos2[:])
            nc.sync.dma_start(ss_sb[:], ss2[:])
            for d0 in range(0, DC, 2):
                nc.sync.dma_start(
                    wqk_sb[:, d0 : d0 + 2, :], wqkT_r[:, d0 : d0 + 2, :]
                )
            for d0 in range(0, DC, 4):
                nc.sync.dma_start(wv_sb[:, d0 : d0 + 4, :], wvT_r[:, d0 : d0 + 4, :])
            nc.sync.dma_start(wo_sb[:], woT_r[:])

            with (
                tc.tile_pool(name="s2t", bufs=3) as s2t,
                tc.tile_pool(name="s2att", bufs=4) as s2att,
                tc.tile_pool(name="s2o", bufs=3) as s2o,
                tc.tile_pool(name="s2nrm", bufs=4) as s2nrm,
                tc.tile_pool(name="ps_acc", bufs=2, space=bass.MemorySpace.PSUM) as ps_acc,
                tc.tile_pool(name="ps_s", bufs=2, space=bass.MemorySpace.PSUM) as ps_s,
                tc.tile_pool(name="ps_av", bufs=2, space=bass.MemorySpace.PSUM) as ps_av,
            ):

                def qk_chunk(etp, c, with_dma):
                    sl = slice(c * XCH, (c + 1) * XCH)
                    if with_dma:
                        for d0 in range(0, DC, 3):
                            nc.sync.dma_start(
                                xsb[:, d0 : d0 + 3, sl], xT_r[:, d0 : d0 + 3, sl]
                            )
                    for half in range(2):  # 0: q, 1: k
                        ps = ps_acc.tile([P, XCH], F32, tag="acc")
                        wcol = etp * 256 + half * P
                        for dc in range(DC):
                            nc.tensor.matmul(
                                ps[:],
                                wqk_sb[:, dc, wcol : wcol + P],
                                xsb[:, dc, sl],
                                start=(dc == 0),
                                stop=(dc == DC - 1),
                            )
                        # rope: PSUM-direct muls (partition-shifted reads
                        # are only legal with a PSUM operand)
                        tcos = s2t.tile([P, XCH], BF16, tag="tcos")
                        trot = s2t.tile([P, XCH], BF16, tag="trot")
                        nc.vector.tensor_mul(tcos[:], ps[:], cos_sb[:, sl])
                        for q_ in range(4):
                            s = (q_ ^ 1) * 32
                            d_ = q_ * 32
                            nc.vector.tensor_mul(
                                trot[d_ : d_ + 32, :],
                                ps[s : s + 32, :],
                                ss_sb[d_ : d_ + 32, sl],
                            )
                        dst = (qT if half == 0 else kT)[:, etp, sl]
                        nc.gpsimd.tensor_add(dst, tcos[:], trot[:])

                def v_chunk(etp, c):
                    # after attention(etp-1, c): ACT copies rank behind that
                    # unit's exps; v1 is only read one unit later
                    for lt2 in range(XCH // P):
                        lk = c * (XCH // P) + lt2
                        lsl = slice(c * XCH + lt2 * P, c * XCH + (lt2 + 1) * P)
                        pv = ps_acc.tile([P, XCH], F32, tag="acc")
                        for dc in range(DC):
                            nc.tensor.matmul(
                                pv[:, 0 : 2 * HD],
                                xsb[:, dc, lsl],
                                wv_sb[:, dc, etp * 2 * HD : (etp + 1) * 2 * HD],
                                start=(dc == 0),
                                stop=(dc == DC - 1),
                            )
                        nc.scalar.copy(
                            v1[:, lk, 2 * etp : 2 * etp + 2, 0:HD],
                            pv[:, 0 : 2 * HD].rearrange("p (h d) -> p h d", h=2),
                        )

                def attention(etp, cq):
                    cqs = slice(cq * ACH, (cq + 1) * ACH)
                    pav0 = ps_av.tile([HD + 1, ACH], F32, tag="av")
                    pav1 = ps_av.tile([HD + 1, ACH], F32, tag="av")
                    atts = {}

                    def emit_av(t):
                        for hh, pav in ((0, pav0), (1, pav1)):
                            for i in range(2):
                                lk = 2 * t + i
                                nc.tensor.matmul(
                                    pav[:],
                                    v1[:, lk, 2 * etp + hh, :],
                                    atts[t][:, i, hh, :],
                                    start=(lk == 0),
                                    stop=(lk == LT - 1),
                                )

                    for t in range(TP):
                        att = s2att.tile([P, 2, 2, ACH], BF16, tag="att")
                        for i in range(2):
                            lk = 2 * t + i
                            psc = ps_s.tile([P, 2 * ACH], F32, tag="s")
                            for hh in range(2):
                                po = hh * HD
                                nc.tensor.matmul(
                                    psc[:, hh * ACH : (hh + 1) * ACH],
                                    kT[po : po + HD, etp, lk * P : (lk + 1) * P],
                                    qT[po : po + HD, etp, cqs],
                                    start=True,
                                    stop=True,
                                )
                            nc.scalar.activation(
                                att[:, i, :, :].rearrange("p h a -> p (h a)"),
                                psc[:],
                                AF.Exp,
                                scale=0.125,
                            )
                        atts[t] = att
                        # av two tpairs behind its exp: PE never waits ScalarE
                        if t > 1:
                            emit_av(t - 2)
                    emit_av(TP - 2)
                    emit_av(TP - 1)

                    # drain pav to SBUF immediately (DVE), freeing the PSUM
                    # bank for the next chunk's av(start); the reciprocal /
                    # broadcast / scale run off the critical path (gpsimd)
                    for hh, pav in ((0, pav0), (1, pav1)):
                        po = hh * HD
                        pavs = s2nrm.tile([HD, ACH], F32, tag="pavs")
                        nc.vector.tensor_copy(pavs[:], pav[0:HD, :])
                        dcp = s2nrm.tile([1, ACH], F32, tag="dcp")
                        nc.vector.tensor_copy(dcp[:], pav[HD : HD + 1, :])
                        rcp = s2nrm.tile([1, ACH], F32, tag="rcp")
                        nc.vector.reciprocal_approx_fast(out=rcp[:], in_=dcp[:])
                        rb = s2nrm.tile([HD, ACH], F32, tag="rb")
                        nc.gpsimd.partition_broadcast(rb[:], rcp[:], channels=HD)
                        nc.vector.tensor_mul(
                            outT[po : po + HD, etp, cqs], pavs[:], rb[:]
                        )

                def oproj(cq):
                    for lt in range(ACH // P):
                        l0 = cq * ACH + lt * P
                        for dn in range(D // ACH):
                            pso = ps_acc.tile([P, ACH], F32, tag="acc")
                            for ec in range(3):
                                nc.tensor.matmul(
                                    pso[:],
                                    outT[:, ec, l0 : l0 + P],
                                    wo_sb[:, ec, dn * ACH : (dn + 1) * ACH],
                                    start=(ec == 0),
                                    stop=(ec == 2),
                                )
                            ot = s2o.tile([P, ACH], F32)
                            nc.vector.tensor_copy(ot[:], pso[:])
                            nc.sync.dma_start(
                                out[l0 : l0 + P, dn * ACH : (dn + 1) * ACH], ot[:]
                            )

                # phase A: qkv+v for pair 0 (x DMA'd once, stays resident)
                for c in range(NCH):
                    qk_chunk(0, c, with_dma=True)
                    v_chunk(0, c)
                # steady state: attention(etp) overlapped with qkv(etp+1)
                for etp in range(3):
                    for cq in range(L // ACH):
                        if etp < 2:
                            qk_chunk(etp + 1, cq, with_dma=False)
                        attention(etp, cq)
                        if etp < 2:
                            v_chunk(etp + 1, cq)
                        if etp == 2:
                            oproj(cq)

                if DEBUG_DUMP:
                    nc.sync.dma_start(
                        dbg_qT[:], qT[:].rearrange("p a b -> p (a b)")
                    )
                    nc.sync.dma_start(
                        dbg_kT[:], kT[:].rearrange("p a b -> p (a b)")
                    )
                    nc.sync.dma_start(
                        dbg_oT[:], outT[:].rearrange("p a b -> p (a b)")
                    )

    nc.compile()
    return nc


_NC_CACHE = None


def _get_nc():
    global _NC_CACHE
    if _NC_CACHE is None:
        _NC_CACHE = build_bass()
    return _NC_CACHE


def make_in_maps(x, w_qkv, w_o, cos, sin):
    x = np.asarray(x, dtype=np.float32)
    w_qkv = np.asarray(w_qkv, dtype=np.float32)
    w_o = np.asarray(w_o, dtype=np.float32)
    cos = np.asarray(cos, dtype=np.float32)
    sin = np.asarray(sin, dtype=np.float32)
    bf16 = ml_dtypes.bfloat16

    cosT = np.ascontiguousarray(cos.T)
    sinT = sin.T
    ss = np.concatenate([-sinT[0:32], sinT[32:64]], axis=0)
    cos2 = np.ascontiguousarray(np.tile(cosT, (2, 1))).astype(np.float16)
    ss2 = np.ascontiguousarray(np.tile(ss, (2, 1))).astype(np.float16)

    def perm_block(w2h):
        return w2h

    in_maps = []
    for c in range(8):
        b, g = c // 4, c % 4
        xTc = np.ascontiguousarray(x[b].T).astype(bf16)
        wq = w_qkv[g * EQ : (g + 1) * EQ]
        wk = w_qkv[D + g * EQ : D + (g + 1) * EQ]
        wv = w_qkv[2 * D + g * EQ : 2 * D + (g + 1) * EQ]
        blocks = []
        for etp in range(3):
            blocks.append(perm_block(wq[etp * 128 : (etp + 1) * 128]))
            blocks.append(perm_block(wk[etp * 128 : (etp + 1) * 128]))
        wqkTc = np.ascontiguousarray(np.concatenate(blocks, 0).T).astype(bf16)
        wvTc = np.ascontiguousarray(wv.T).astype(bf16)
        woTc = np.ascontiguousarray(w_o[:, g * EQ : (g + 1) * EQ].T).astype(bf16)
        in_maps.append(
            {
                "xT": xTc,
                "wqkT": wqkTc,
                "wvT": wvTc,
                "woT": woTc,
                "cos2": cos2,
                "ss2": ss2,
            }
        )
    return in_maps


def kernel(x, w_qkv, w_o, cos, sin):
    nc = _get_nc()
    in_maps = make_in_maps(x, w_qkv, w_o, cos, sin)
    res = run_bass_kernel_spmd(nc, in_maps, core_ids=list(range(8)))
    outs = [res.results[c]["out"] for c in range(8)]
    full = np.stack(
        [
            outs[0] + outs[1] + outs[2] + outs[3],
            outs[4] + outs[5] + outs[6] + outs[7],
        ]
    ).astype(np.float32)
    return full


# revision 34
# speedup vs baseline: 1.2931x; 1.0115x over previous
"""v8: all-bf16 matmuls, permuted head layout, pipelined attention.

vs v6 fp32r baseline (444us):
- All matmul operands bf16: separate LDWEIGHTS (FWL + PE pull-ahead) vs
  fp32r's serial self-loading; row-disjoint matmuls run concurrently on
  the PE sub-arrays.
- q/k feature rows permuted to [h0.lo | h1.lo | h0.hi | h1.hi] (32 each):
  rope's rotate-half becomes 2 DVE muls (+-64 partition shift) instead of
  4, and scores run as 4 concurrent K=32 row-tiled matmuls per k-tile.
- V stored [128, ktile, head, 1+64] with the softmax-denominator ones
  column FIRST, so the reciprocal chain reads partition 0 (no shifts).
- x converted to bf16 on host, streamed once, kept resident in SBUF.
- exp owns ScalarE (~200us floor); attention av matmuls trail their exp
  by 2 k-tile-pairs; pav is copied to SBUF immediately after the last av
  so the next chunk's av(start) isn't gated on the normalize chain.
"""

import os
import sys

for _p in ("/opt/trn_rl_repo", "/root/.axon_site/_ro/trn_rl_repo"):
    if os.path.isdir(_p) and _p not in sys.path:
        sys.path.insert(0, _p)

import contextlib

import ml_dtypes
import numpy as np

import concourse.bass as bass
import concourse.tile as tile
from concourse import bacc, mybir
from concourse.bass_utils import run_bass_kernel_spmd

P = 128
L = 2048
D = 1536
HL = 6           # heads per core
HD = 64
EQ = 384         # HL * HD
DC = D // P      # 12
LT = L // P      # 16
TP = LT // 2     # 8 k-tile pairs
ACH = 512        # attention q chunk
XCH = 512        # qkv l chunk
NCH = L // XCH   # 4
F32 = mybir.dt.float32
BF16 = mybir.dt.bfloat16
F16 = mybir.dt.float16
AF = mybir.ActivationFunctionType

DEBUG_DUMP = os.environ.get("DEBUG_DUMP", "0") == "1"


def build_bass(repeat=1):
    nc = bacc.Bacc("TRN2", target_bir_lowering=False, debug=False, num_devices=8)
    xT = nc.dram_tensor("xT", [D, L], BF16, kind="ExternalInput")
    wqkT = nc.dram_tensor("wqkT", [D, 768], BF16, kind="ExternalInput")
    wvT = nc.dram_tensor("wvT", [D, EQ], BF16, kind="ExternalInput")
    woT = nc.dram_tensor("woT", [EQ, D], BF16, kind="ExternalInput")
    cos2 = nc.dram_tensor("cos2", [P, L], F16, kind="ExternalInput")
    ss2 = nc.dram_tensor("ss2", [P, L], F16, kind="ExternalInput")
    out = nc.dram_tensor("out", [L, D], F32, kind="ExternalOutput")
    if DEBUG_DUMP:
        dbg_qT = nc.dram_tensor("dbg_qT", [P, 3 * L], BF16, kind="ExternalOutput")
        dbg_kT = nc.dram_tensor("dbg_kT", [P, 3 * L], BF16, kind="ExternalOutput")
        dbg_oT = nc.dram_tensor("dbg_oT", [P, 3 * L], BF16, kind="ExternalOutput")

    xT_r = xT.rearrange("(dc p) l -> p dc l", p=P)
    wqkT_r = wqkT.rearrange("(dc p) e -> p dc e", p=P)
    wvT_r = wvT.rearrange("(dc p) e -> p dc e", p=P)
    woT_r = woT.rearrange("(ec p) d -> p ec d", p=P)

    with tile.TileContext(nc) as tc:
        rep_cm = tc.For_i(0, repeat, 1) if repeat > 1 else contextlib.nullcontext()
        with rep_cm, tc.tile_pool(name="persist", bufs=1) as persist:
            qT = persist.tile([P, 3, L], BF16)
            kT = persist.tile([P, 3, L], BF16)
            outT = persist.tile([P, 3, L], BF16)
            # [P, ktile, head, 64 dims + ones (softmax denominator)]
            v1 = persist.tile([P, LT, HL, HD + 1], BF16)
            xsb = persist.tile([P, DC, L], BF16)
            cos_sb = persist.tile([P, L], F16)
            ss_sb = persist.tile([P, L], F16)
            wqk_sb = persist.tile([P, DC, 768], BF16)
            wv_sb = persist.tile([P, DC, EQ], BF16)
            wo_sb = persist.tile([P, 3, D], BF16)

            ones_c = nc.const_aps.tensor(1.0, (P, 1), F32)
            nc.vector.tensor_copy(
                v1[:, :, :, HD : HD + 1], ones_c.to_broadcast([P, LT, HL, 1])
            )
            nc.sync.dma_start(cos_sb[:], cos2[:])
            nc.sync.dma_start(ss_sb[:], ss2[:])
            for d0 in range(0, DC, 2):
                nc.sync.dma_start(
                    wqk_sb[:, d0 : d0 + 2, :], wqkT_r[:, d0 : d0 + 2, :]
                )
            for d0 in range(0, DC, 4):
                nc.sync.dma_start(wv_sb[:, d0 : d0 + 4, :], wvT_r[:, d0 : d0 + 4, :])
            nc.sync.dma_start(wo_sb[:], woT_r[:])

            with (
                tc.tile_pool(name="s2t", bufs=3) as s2t,
                tc.tile_pool(name="s2att", bufs=4) as s2att,
                tc.tile_pool(name="s2o", bufs=3) as s2o,
                tc.tile_pool(name="s2nrm", bufs=4) as s2nrm,
                tc.tile_pool(name="ps_acc", bufs=2, space=bass.MemorySpace.PSUM) as ps_acc,
                tc.tile_pool(name="ps_s", bufs=2, space=bass.MemorySpace.PSUM) as ps_s,
                tc.tile_pool(name="ps_av", bufs=2, space=bass.MemorySpace.PSUM) as ps_av,
            ):

                def qk_chunk(etp, c, with_dma):
                    sl = slice(c * XCH, (c + 1) * XCH)
                    if with_dma:
                        for d0 in range(0, DC, 3):
                            nc.sync.dma_start(
                                xsb[:, d0 : d0 + 3, sl], xT_r[:, d0 : d0 + 3, sl]
                            )
                    for half in range(2):  # 0: q, 1: k
                        ps = ps_acc.tile([P, XCH], F32, tag="acc")
                        wcol = etp * 256 + half * P
                        for dc in range(DC):
                            nc.tensor.matmul(
                                ps[:],
                                wqk_sb[:, dc, wcol : wcol + P],
                                xsb[:, dc, sl],
                                start=(dc == 0),
                                stop=(dc == DC - 1),
                            )
                        # rope: PSUM-direct muls (partition-shifted reads
                        # are only legal with a PSUM operand)
                        tcos = s2t.tile([P, XCH], BF16, tag="tcos")
                        trot = s2t.tile([P, XCH], BF16, tag="trot")
                        nc.vector.tensor_mul(tcos[:], ps[:], cos_sb[:, sl])
                        for q_ in range(4):
                            s = (q_ ^ 1) * 32
                            d_ = q_ * 32
                            nc.vector.tensor_mul(
                                trot[d_ : d_ + 32, :],
                                ps[s : s + 32, :],
                                ss_sb[d_ : d_ + 32, sl],
                            )
                        dst = (qT if half == 0 else kT)[:, etp, sl]
                        nc.gpsimd.tensor_add(dst, tcos[:], trot[:])

                def v_chunk(etp, c):
                    # after attention(etp-1, c): ACT copies rank behind that
                    # unit's exps; v1 is only read one unit later
                    for lt2 in range(XCH // P):
                        lk = c * (XCH // P) + lt2
                        lsl = slice(c * XCH + lt2 * P, c * XCH + (lt2 + 1) * P)
                        pv = ps_acc.tile([P, XCH], F32, tag="acc")
                        for dc in range(DC):
                            nc.tensor.matmul(
                                pv[:, 0 : 2 * HD],
                                xsb[:, dc, lsl],
                                wv_sb[:, dc, etp * 2 * HD : (etp + 1) * 2 * HD],
                                start=(dc == 0),
                                stop=(dc == DC - 1),
                            )
                        nc.scalar.copy(
                            v1[:, lk, 2 * etp : 2 * etp + 2, 0:HD],
                            pv[:, 0 : 2 * HD].rearrange("p (h d) -> p h d", h=2),
                        )

                def attention(etp, cq):
                    cqs = slice(cq * ACH, (cq + 1) * ACH)
                    pav0 = ps_av.tile([HD + 1, ACH], F32, tag="av")
                    pav1 = ps_av.tile([HD + 1, ACH], F32, tag="av")
                    atts = {}

                    def emit_av(t):
                        for hh, pav in ((0, pav0), (1, pav1)):
                            for i in range(2):
                                lk = 2 * t + i
                                nc.tensor.matmul(
                                    pav[:],
                                    v1[:, lk, 2 * etp + hh, :],
                                    atts[t][:, i, hh, :],
                                    start=(lk == 0),
                                    stop=(lk == LT - 1),
                                )

                    for t in range(TP):
                        att = s2att.tile([P, 2, 2, ACH], BF16, tag="att")
                        for i in range(2):
                            lk = 2 * t + i
                            psc = ps_s.tile([P, 2 * ACH], F32, tag="s")
                            for hh in range(2):
                                po = hh * HD
                                nc.tensor.matmul(
                                    psc[:, hh * ACH : (hh + 1) * ACH],
                                    kT[po : po + HD, etp, lk * P : (lk + 1) * P],
                                    qT[po : po + HD, etp, cqs],
                                    start=True,
                                    stop=True,
                                )
                            nc.scalar.activation(
                                att[:, i, :, :].rearrange("p h a -> p (h a)"),
                                psc[:],
                                AF.Exp,
                                scale=0.125,
                            )
                        atts[t] = att
                        # av two tpairs behind its exp: PE never waits ScalarE
                        if t > 1:
                            emit_av(t - 2)
                    emit_av(TP - 2)
                    emit_av(TP - 1)

                    # drain pav to SBUF immediately (DVE), freeing the PSUM
                    # bank for the next chunk's av(start); the reciprocal /
                    # broadcast / scale are DEFERRED one unit so they never
                    # head-of-line-block the DVE FIFO in front of rope
                    drains = []
                    for hh, pav in ((0, pav0), (1, pav1)):
                        pavs = s2nrm.tile([HD, ACH], F32, tag="pavs")
                        nc.vector.tensor_copy(pavs[:], pav[0:HD, :])
                        dcp = s2nrm.tile([1, ACH], F32, tag="dcp")
                        nc.vector.tensor_copy(dcp[:], pav[HD : HD + 1, :])
                        drains.append((hh, pavs, dcp))
                    return (etp, cqs, drains)

                def normalize(pending):
                    etp, cqs, drains = pending
                    for hh, pavs, dcp in drains:
                        po = hh * HD
                        rcp = s2nrm.tile([1, ACH], F32, tag="rcp")
                        nc.vector.reciprocal_approx_fast(out=rcp[:], in_=dcp[:])
                        rb = s2nrm.tile([HD, ACH], F32, tag="rb")
                        nc.gpsimd.partition_broadcast(rb[:], rcp[:], channels=HD)
                        nc.vector.tensor_mul(
                            outT[po : po + HD, etp, cqs], pavs[:], rb[:]
                        )

                def oproj(cq):
                    for lt in range(ACH // P):
                        l0 = cq * ACH + lt * P
                        for dn in range(D // ACH):
                            pso = ps_acc.tile([P, ACH], F32, tag="acc")
                            for ec in range(3):
                                nc.tensor.matmul(
                                    pso[:],
                                    outT[:, ec, l0 : l0 + P],
                                    wo_sb[:, ec, dn * ACH : (dn + 1) * ACH],
                                    start=(ec == 0),
                                    stop=(ec == 2),
                                )
                            ot = s2o.tile([P, ACH], F32)
                            nc.vector.tensor_copy(ot[:], pso[:])
                            nc.sync.dma_start(
                                out[l0 : l0 + P, dn * ACH : (dn + 1) * ACH], ot[:]
                            )

                # phase A: qkv+v for pair 0 (x DMA'd once, stays resident)
                for c in range(NCH):
                    qk_chunk(0, c, with_dma=True)
                    v_chunk(0, c)
                # steady state: attention(etp) overlapped with qkv(etp+1);
                # normalize of unit u runs inside unit u+1, oproj one more
                # unit later (etp2 only)
                pending_norm = None
                pending_oproj = []
                for etp in range(3):
                    for cq in range(L // ACH):
                        if etp < 2:
                            qk_chunk(etp + 1, cq, with_dma=False)
                        pn = attention(etp, cq)
                        if pending_norm is not None:
                            normalize(pending_norm)
                            if pending_norm[0] == 2:
                                pending_oproj.append(pending_norm[1])
                        pending_norm = pn
                        if etp < 2:
                            v_chunk(etp + 1, cq)
                        if pending_oproj:
                            oproj(pending_oproj.pop(0).start // ACH)
                normalize(pending_norm)
                oproj(3)

                if DEBUG_DUMP:
                    nc.sync.dma_start(
                        dbg_qT[:], qT[:].rearrange("p a b -> p (a b)")
                    )
                    nc.sync.dma_start(
                        dbg_kT[:], kT[:].rearrange("p a b -> p (a b)")
                    )
                    nc.sync.dma_start(
                        dbg_oT[:], outT[:].rearrange("p a b -> p (a b)")
                    )

    nc.compile()
    return nc


_NC_CACHE = None


def _get_nc():
    global _NC_CACHE
    if _NC_CACHE is None:
        _NC_CACHE = build_bass()
    return _NC_CACHE


def make_in_maps(x, w_qkv, w_o, cos, sin):
    x = np.asarray(x, dtype=np.float32)
    w_qkv = np.asarray(w_qkv, dtype=np.float32)
    w_o = np.asarray(w_o, dtype=np.float32)
    cos = np.asarray(cos, dtype=np.float32)
    sin = np.asarray(sin, dtype=np.float32)
    bf16 = ml_dtypes.bfloat16

    cosT = np.ascontiguousarray(cos.T)
    sinT = sin.T
    ss = np.concatenate([-sinT[0:32], sinT[32:64]], axis=0)
    cos2 = np.ascontiguousarray(np.tile(cosT, (2, 1))).astype(np.float16)
    ss2 = np.ascontiguousarray(np.tile(ss, (2, 1))).astype(np.float16)

    def perm_block(w2h):
        return w2h

    in_maps = []
    for c in range(8):
        b, g = c // 4, c % 4
        xTc = np.ascontiguousarray(x[b].T).astype(bf16)
        wq = w_qkv[g * EQ : (g + 1) * EQ]
        wk = w_qkv[D + g * EQ : D + (g + 1) * EQ]
        wv = w_qkv[2 * D + g * EQ : 2 * D + (g + 1) * EQ]
        blocks = []
        for etp in range(3):
            blocks.append(perm_block(wq[etp * 128 : (etp + 1) * 128]))
            blocks.append(perm_block(wk[etp * 128 : (etp + 1) * 128]))
        wqkTc = np.ascontiguousarray(np.concatenate(blocks, 0).T).astype(bf16)
        wvTc = np.ascontiguousarray(wv.T).astype(bf16)
        woTc = np.ascontiguousarray(w_o[:, g * EQ : (g + 1) * EQ].T).astype(bf16)
        in_maps.append(
            {
                "xT": xTc,
                "wqkT": wqkTc,
                "wvT": wvTc,
                "woT": woTc,
                "cos2": cos2,
                "ss2": ss2,
            }
        )
    return in_maps


def kernel(x, w_qkv, w_o, cos, sin):
    nc = _get_nc()
    in_maps = make_in_maps(x, w_qkv, w_o, cos, sin)
    res = run_bass_kernel_spmd(nc, in_maps, core_ids=list(range(8)))
    outs = [res.results[c]["out"] for c in range(8)]
    full = np.stack(
        [
            outs[0] + outs[1] + outs[2] + outs[3],
            outs[4] + outs[5] + outs[6] + outs[7],
        ]
    ).astype(np.float32)
    return full
